# revision 31
# baseline (speedup 1.0000x reference)
"""CvT attention block (depthwise conv QKV + MHA) on 8 Trainium2 NeuronCores,
data-parallel over batch.

v5 (head-pair packed attention + DVE-assisted softmax exp):
  - Phase A/B as v4: convs as diagonal-weight PE matmuls, projections bf16.
    KT zero-padded to 7x128 kv-tiles so every QK LDWEIGHTS is a 128-col FWL
    load; V-hat drops the ones-column (denominators via M=1 matmuls).
  - Phase C processes head PAIRS at l-chunks of 256: QK row-tiled
    (head h on PE rows 0-63, h+1 on 64-127, concurrent), AV col-tiled
    (head h -> PSUM partitions 0-63, h+1 -> 64-127, concurrent), so the
    128x128 array is fully used despite d=64 heads.
  - softmax exp is split across TWO engines: ACT exps S[:, 0:2048]
    (kv-tiles 0-3); a custom 2-instruction DVE op exps S[:, 2048:3584]
    (kv-tiles 4-6): p = deg-3 Taylor of e^(z*SCALE/64), then p^64 via 6
    squarings (~1e-4 poly err, bf16-out bound).
  - PSUM: one [128,3584] S tile (7 banks, ring via bufs=1) + one shared
    bank cycling AV-accum/denoms -> rb -> out-proj windows. 1-block
    software pipeline: block i emits QK(i), AV(i-1), rb/norm(i-2),
    out-proj/store pieces of the previous chunk, keeping the PE dense so
    the HAM clock gate stays at 8/8.
"""

import contextlib
import numpy as np
import ml_dtypes
from concourse import mybir
import concourse.bacc as bacc
import concourse.tile as tile
from concourse.bass_utils import run_bass_kernel_spmd
from concourse.dve_ops import DveOp, OPS, CUSTOM_DVE_SPECS, _SUB_OPCODE_FOR_NAME
from concourse.dve_spec import Spec, Src0, C0, C1, C2, One, sq, lower
from concourse.dve_uop import DveOpSpec

F32 = mybir.dt.float32
BF16 = mybir.dt.bfloat16
AFT = mybir.ActivationFunctionType

C = 384
T = 3136            # 56*56
TKV = 784           # 28*28
TKVP = 896          # 7*128 (zero-padded)
NH = 6
SCALE = C ** (-0.5)
EPS = 1e-5
XB = 3368           # 2 + 58*58 + 8 slack; data (r,c) at 2 + (1+r)*58 + 1+c

LS = 128
LC = [(i * LS, min(LS, T - i * LS)) for i in range((T + LS - 1) // LS)]  # 13
TT = [(j * 128, min(128, TKV - j * 128)) for j in range(7)]  # kv tiles
TAPS = [(t // 3 - 1, t % 3 - 1) for t in range(9)]

_CACHE = {}


# ---- custom DVE exp: exp(x*SCALE) = (deg3 e^(x*SCALE/64))^64 ---------------
def _register_dve(op_name, body, reference):
    spec = Spec(body=body, reference=reference)
    shas = {}
    for ver in ("v3", "v4"):
        uops = lower(spec, ver=ver)
        shas[ver] = DveOpSpec(name=op_name, opcode=0, uops=uops,
                              rd1_en=False).sha(ver)
    op = DveOp(op_name, spec, subdim=False, uops_sha=shas)
    if op_name not in _SUB_OPCODE_FOR_NAME:
        OPS.append(op)
        CUSTOM_DVE_SPECS[op_name] = spec
        _SUB_OPCODE_FOR_NAME[op_name] = max(_SUB_OPCODE_FOR_NAME.values()) + 1
        assert _SUB_OPCODE_FOR_NAME[op_name] < 0x20
    return op


_u = Src0 * C0
EXP_P1 = _register_dve(
    "ANT_EXP_P1", ((_u * C1 + One) * _u * C2 + One) * _u + One,
    lambda in0, in1, s0, s1, imm2: (
        ((in0 * s0 * s1 + 1.0) * (in0 * s0) * imm2 + 1.0) * (in0 * s0) + 1.0),
)
EXP_P2 = _register_dve(
    "ANT_EXP_P2", sq(sq(sq(sq(sq(sq(Src0)))))),
    lambda in0, in1, s0, s1, imm2: in0 ** 64,
)


def _emit(nc, tc, ctx, d, reps):
    pers = ctx.enter_context(tc.tile_pool(name="pers", bufs=1))

    wq = [pers.tile([128, C], BF16, tag=f"wq{i}", name=f"wq{i}") for i in range(3)]
    wk = [pers.tile([128, C], BF16, tag=f"wk{i}", name=f"wk{i}") for i in range(3)]
    wvp = [pers.tile([128, NH * 65], BF16, tag=f"wvp{i}", name=f"wvp{i}")
           for i in range(3)]
    wpj = [pers.tile([128, C], BF16, tag=f"wpj{i}", name=f"wpj{i}")
           for i in range(3)]
    wd = [pers.tile([128, 27 * 128], BF16, tag=f"wd{i}", name=f"wd{i}")
          for i in range(3)]
    ind2 = pers.tile([2, 128], BF16, tag="ind2", name="ind2")
    wb = [pers.tile([128, 30], F32, tag=f"wb{i}", name=f"wb{i}")
          for i in range(3)]
    bpjW = pers.tile([128, 1024], F32, tag="bpjW", name="bpjW")
    QT = [pers.tile([128, T], BF16, tag=f"QT{i}", name=f"QT{i}") for i in range(3)]
    KT = [pers.tile([128, TKVP], BF16, tag=f"KT{i}", name=f"KT{i}")
          for i in range(3)]
    Vh = [pers.tile([128, NH * 65], BF16, tag=f"Vh{i}", name=f"Vh{i}")
          for i in range(7)]

    nc.sync.dma_start(wd[0][:], d["wd"][0])
    nc.sync.dma_start(wb[0][:], d["wb"][0])
    for i in range(3):
        nc.vector.memset(KT[i][:, TKV:TKVP], 0.0)

    def _late_dmas():
        for i in range(1, 3):
            nc.sync.dma_start(wd[i][:], d["wd"][i])
            nc.sync.dma_start(wb[i][:], d["wb"][i])
        for i in range(3):
            nc.sync.dma_start(wq[i][:], d["wq"][i * 128:(i + 1) * 128, :])
            nc.sync.dma_start(wk[i][:], d["wk"][i * 128:(i + 1) * 128, :])
            nc.sync.dma_start(wvp[i][:], d["wvp"][i * 128:(i + 1) * 128, :])
            nc.sync.dma_start(wpj[i][:], d["wpj"][i * 128:(i + 1) * 128, :])
        nc.sync.dma_start(ind2[:], d["ind2"])
        nc.sync.dma_start(bpjW[:], d["bpjW"])

    for rep in range(reps):
        sfx = f"r{rep}"
        with contextlib.ExitStack() as ph:
            yp = ph.enter_context(tc.tile_pool(name="y" + sfx, bufs=1))
            yq = [yp.tile([128, T], BF16, tag=f"yq{i}", name=f"yq{i}")
                  for i in range(3)]
            yk = [yp.tile([128, TKV], BF16, tag=f"yk{i}", name=f"yk{i}")
                  for i in range(3)]
            yv = [yp.tile([128, TKV], BF16, tag=f"yv{i}", name=f"yv{i}")
                  for i in range(3)]
            xbs = [yp.tile([128, XB], BF16, tag=f"xb{i}", name=f"xb{i}")
                   for i in range(3)]
            ps = ph.enter_context(
                tc.tile_pool(name="ps" + sfx, bufs=1, space="PSUM"))
            # 4 + 1 + 1 + 1 banks; one spare
            Sb = ps.tile([128, 2048], F32, tag="Sb", name="Sb")
            rw = ps.tile([128, 512], F32, tag="rw", name="rw")
            pcv = ps.tile([128, 512], F32, tag="pcv", name="pcv")
            av = ps.tile([128, 512], F32, tag="av", name="av")
            psB0 = ps.tile([128, 512], F32, tag="psB0", name="psB0")

            def next_psB():
                return psB0
            cw = ph.enter_context(tc.tile_pool(name="cw" + sfx, bufs=2))

            for ch in range(3):
                nc.sync.dma_start(xbs[ch][:], d["xb"][ch * 128:(ch + 1) * 128, :])
                if rep == 0 and ch == 0:
                    _late_dmas()

            # ---- conv/proj piece emitters (PE work, dependency-light) ----
            def conv_q(ch, k):
                """q-conv rows 8k..8k+8 for channel block ch -> yq[ch]."""
                xb = xbs[ch]
                base = 2 + (1 + 8 * k) * 58
                for t, (di, dj) in enumerate(TAPS):
                    nc.tensor.matmul(pcv[:, 0:464],
                                     wd[ch][:, t * 128:(t + 1) * 128],
                                     xb[:, base + 58 * di + dj:
                                        base + 58 * di + dj + 464],
                                     start=(t == 0), stop=(t == 8))
                src = pcv[:, 0:464].rearrange(
                    "p (r c) -> p r c", c=58)[:, :, 1:57]
                dst = yq[ch][:, 448 * k:448 * (k + 1)].rearrange(
                    "p (r c) -> p r c", c=56)
                nc.scalar.activation(dst, src, AFT.Identity,
                                     bias=wb[ch][:, 27:28])

            def conv_kv(ch, cv, r0):
                xb = xbs[ch]
                x3 = xb[:, 2:2 + 3364].rearrange("p (r c) -> p r c", c=58)
                ykv = (yk if cv == 1 else yv)[ch]
                for t, (di, dj) in enumerate(TAPS):
                    mv = x3[:, 1 + 2 * r0 + di:1 + 2 * r0 + di + 28:2,
                            1 + dj:1 + dj + 56:2]
                    nc.tensor.matmul(
                        pcv[:, 0:392],
                        wd[ch][:, (9 * cv + t) * 128:(9 * cv + t + 1) * 128],
                        mv, start=(t == 0), stop=(t == 8))
                nc.scalar.activation(
                    ykv[:, r0 * 28:r0 * 28 + 392], pcv[:, 0:392], AFT.Identity,
                    bias=wb[ch][:, 27 + cv:28 + cv])

            def proj_q(co, g):
                """Q projection for output block co, l-cols [512g, 512g+ls)."""
                lo = 512 * g
                ls = min(512, T - lo)
                psB = next_psB()
                for ch in range(3):
                    nc.tensor.matmul(psB[:, 0:ls],
                                     wq[ch][:, co * 128:(co + 1) * 128],
                                     yq[ch][:, lo:lo + ls],
                                     start=(ch == 0), stop=(ch == 2))
                nc.scalar.activation(QT[co][:, lo:lo + ls], psB[:, 0:ls],
                                     AFT.Copy)

            def proj_k(co, g):
                to = 512 * g
                ts = min(512, TKV - to)
                psB = next_psB()
                for ch in range(3):
                    nc.tensor.matmul(psB[:, 0:ts],
                                     wk[ch][:, co * 128:(co + 1) * 128],
                                     yk[ch][:, to:to + ts],
                                     start=(ch == 0), stop=(ch == 2))
                nc.scalar.activation(KT[co][:, to:to + ts], psB[:, 0:ts],
                                     AFT.Copy)

            def proj_v(ti):
                to, ts = TT[ti]
                psB = next_psB()
                for ch in range(3):
                    nc.tensor.matmul(psB[0:ts, 0:NH * 65],
                                     yv[ch][:, to:to + ts], wvp[ch][:],
                                     start=(ch == 0), stop=(ch == 2))
                nc.scalar.activation(Vh[ti][0:ts, :], psB[0:ts, 0:NH * 65],
                                      AFT.Copy)
                nc.vector.memset(Vh[ti][0:ts, 64:NH * 65:65], 1.0)

            # ---- prologue: k/v side + enough of the q side for sb 0-11 ----
            for ch in range(3):
                for cv in (1, 2):
                    for r0 in (0, 14):
                        conv_kv(ch, cv, r0)
            for co in range(3):
                for g in range(2):
                    proj_k(co, g)
            for ti in range(7):
                proj_v(ti)
            for k in (0, 1):
                for ch in range(3):
                    conv_q(ch, k)
            for co in range(3):
                proj_q(co, 0)

            # remaining conv/proj work, popped one piece per 2 superblocks
            fill = []
            nextg = [1, 1, 1]
            for k in range(2, 7):
                for ch in range(3):
                    fill.append(lambda ch=ch, k=k: conv_q(ch, k))
                gr = (448 * (k + 1)) // 512
                for co in range(3):
                    while nextg[co] < min(gr, 7):
                        g = nextg[co]
                        fill.append(lambda co=co, g=g: proj_q(co, g))
                        nextg[co] += 1
            for co in range(3):
                while nextg[co] < 7:
                    g = nextg[co]
                    fill.append(lambda co=co, g=g: proj_q(co, g))
                    nextg[co] += 1

            # ---- phase C superblocks: (chunk, pair), AV lagging 1 block ----
            def qk_block(c, p):
                lo, ls = LC[c]
                for j in range(7):
                    for ho in range(2):
                        o = 896 * ho + 128 * j
                        nc.tensor.matmul(
                            Sb[0:128, o:o + ls],
                            KT[p][64 * ho:64 * ho + 64, 128 * j:128 * (j + 1)],
                            QT[p][64 * ho:64 * ho + 64, lo:lo + ls],
                            start=True, stop=True)
                et = cw.tile([128, 1792], BF16, tag="et", name="et")
                nc.scalar.activation(et[:], Sb[:, 0:1792], AFT.Exp,
                                     scale=float(SCALE))
                return {"c": c, "p": p, "et": et}

            def av_block(st):
                c, p = st["c"], st["p"]
                lo, ls = LC[c]
                OTb, rcfC = st["OTb_t"], st["rcfC_t"]
                et = st["et"]
                for ho in range(2):
                    h = 2 * p + ho
                    for j, (to, ts) in enumerate(TT):
                        nc.tensor.matmul(
                            av[0:65, 128 * ho:128 * ho + ls],
                            Vh[j][0:ts, 65 * h:65 * h + 65],
                            et[0:ts, 896 * ho + 128 * j:896 * ho + 128 * j + ls],
                            start=(j == 0), stop=(j == 6))
                for ho in range(2):
                    nc.scalar.activation(
                        OTb[64 * ho:64 * ho + 64, 128 * p:128 * p + ls],
                        av[0:64, 128 * ho:128 * ho + ls], AFT.Copy)
                nc.scalar.activation(rcfC[0:1, 256 * p:256 * p + 256],
                                     av[64:65, 0:256], AFT.Copy)

            def tail_enqueue(tq, c, OTb, rcfC):
                lo, ls = LC[c]
                lsz = min(128, ls)
                rcA = cw.tile([1, 768], F32, tag="rcA", name="rcA")
                rc2r = cw.tile([1, 768], BF16, tag="rc2r", name="rc2r")
                rc2v = cw.tile([2, 384], BF16, tag="rc2v", name="rc2v")
                osb = cw.tile([128, 512], F32, tag="osb", name="osb")

                def s_recip():
                    nc.vector.reciprocal_approx_fast(rcA[:], rcfC[:])
                    with nc.allow_low_precision(reason="bf16 softmax recip"):
                        nc.vector.tensor_copy(rc2r[:], rcA[:])
                    r3 = rc2r[0:1, :].rearrange("a (p h l) -> a p h l",
                                                h=2, l=128)
                    d3 = rc2v[:].rearrange("p (q l) -> p q l", l=128)
                    for ho in range(2):
                        nc.sync.dma_start(d3[ho:ho + 1], r3[:, :, ho, :])

                def s_norm(p):
                    nc.tensor.matmul(rw[:, 0:ls], ind2[:],
                                     rc2v[0:2, 128 * p:128 * p + ls],
                                     start=True, stop=True)
                    nc.vector.tensor_mul(OTb[:, 128 * p:128 * p + ls],
                                         OTb[:, 128 * p:128 * p + ls],
                                         rw[:, 0:ls])

                def s_win(cs):
                    w = rw[0:128, 128 * (cs + 1):128 * (cs + 2)]
                    for ch in range(3):
                        nc.tensor.matmul(
                            w[0:lsz, 0:128],
                            OTb[:, 128 * ch:128 * ch + lsz],
                            wpj[ch][:, 128 * cs:128 * cs + 128],
                            start=(ch == 0), stop=(ch == 2))
                    nc.vector.tensor_add(
                        osb[0:lsz, 128 * cs:128 * cs + 128],
                        w[0:lsz, 0:128],
                        bpjW[0:lsz, 128 * cs:128 * cs + 128])

                def s_store():
                    nc.sync.dma_start(d["out"][lo:lo + ls, :], osb[0:ls, 0:C])

                tq.append(s_recip)
                for p in range(3):
                    tq.append(lambda p=p: s_norm(p))

                def s_w01():
                    s_win(0)
                    s_win(1)

                def s_w2():
                    s_win(2)
                    s_store()
                tq.append(s_w01)
                tq.append(s_w2)

            blocks = [(c, p) for c in range(len(LC)) for p in range(3)]
            prev = None
            tq = []
            OTb = rcfC = None
            for i, (c, p) in enumerate(blocks):
                if p == 0:
                    OTb = cw.tile([128, 384], BF16, tag="OTb", name="OTb",
                                  bufs=3)
                    rcfC = cw.tile([1, 768], F32, tag="rcfC", name="rcfC",
                                   bufs=3)
                st = qk_block(c, p)
                st["OTb_t"], st["rcfC_t"] = OTb, rcfC
                if tq:
                    tq.pop(0)()
                if prev is not None:
                    av_block(prev)
                    if prev["p"] == 2:
                        tail_enqueue(tq, prev["c"], prev["OTb_t"],
                                     prev["rcfC_t"])
                if tq:
                    tq.pop(0)()
                if i % 2 == 0 and fill:
                    fill.pop(0)()
                prev = st

            av_block(prev)
            tail_enqueue(tq, prev["c"], prev["OTb_t"], prev["rcfC_t"])
            for s in fill:
                s()
            for s in tq:
                s()


def _build(reps=1):
    if reps in _CACHE:
        return _CACHE[reps]
    nc = bacc.Bacc("TRN2", target_bir_lowering=False, debug=False)
    d = {
        "xb": nc.dram_tensor("xb", [C, XB], BF16, kind="ExternalInput").ap(),
        "wb": nc.dram_tensor("wb", [3, 128, 30], F32, kind="ExternalInput").ap(),
        "wd": nc.dram_tensor("wd", [3, 128, 27 * 128], BF16,
                             kind="ExternalInput").ap(),
        "wq": nc.dram_tensor("wq", [C, C], BF16, kind="ExternalInput").ap(),
        "wk": nc.dram_tensor("wk", [C, C], BF16, kind="ExternalInput").ap(),
        "wvp": nc.dram_tensor("wvp", [C, NH * 65], BF16,
                              kind="ExternalInput").ap(),
        "wpj": nc.dram_tensor("wpj", [C, C], BF16, kind="ExternalInput").ap(),
        "ind2": nc.dram_tensor("ind2", [2, 128], BF16,
                               kind="ExternalInput").ap(),
        "bpjW": nc.dram_tensor("bpjW", [128, 1024], F32,
                               kind="ExternalInput").ap(),
        "out": nc.dram_tensor("out", [T, C], F32, kind="ExternalOutput").ap(),
    }
    with tile.TileContext(nc) as tc:
        with contextlib.ExitStack() as ctx:
            _emit(nc, tc, ctx, d, reps)
    nc.compile()
    _CACHE[reps] = nc
    return nc


def _bpjw(bproj):
    w = np.zeros((128, 1024), np.float32)
    for k in range(2):
        w[:, k * 512:k * 512 + C] = bproj[None, :]
    return w


def _host_prep(x, conv_q, conv_k, conv_v, bn_q, bn_k, bn_v, Wq, Wk, Wv,
               Wproj, bproj):
    bf = ml_dtypes.bfloat16
    B = x.shape[0]
    x = np.asarray(x, np.float32)
    # 58x58 zero-padded bf16 image: data (r,c) at col 2 + (1+r)*58 + 1+c
    xb = np.zeros((B, C, XB), bf)
    xi = np.ascontiguousarray(x.transpose(0, 2, 1)).reshape(B, C, 56, 56)
    xb3 = xb[:, :, 2:2 + 3364].reshape(B, C, 58, 58)
    xb3[:, :, 1:57, 1:57] = xi.astype(bf)

    wb = np.zeros((3, 128, 30), np.float32)
    whs = []
    for cv, (w, bn) in enumerate(((conv_q, bn_q), (conv_k, bn_k),
                                  (conv_v, bn_v))):
        g, b, m, v = [np.asarray(bn[i], np.float64) for i in range(4)]
        a = g / np.sqrt(v + EPS)
        bias = (b - m * a).astype(np.float32)
        wh = (np.asarray(w, np.float64).reshape(C, 9) * a[:, None]).astype(
            np.float32)
        whs.append(wh)
        for ch in range(3):
            wb[ch, :, 9 * cv:9 * cv + 9] = wh[ch * 128:(ch + 1) * 128]
            wb[ch, :, 27 + cv] = bias[ch * 128:(ch + 1) * 128]

    # diag-packed conv weights for the PE:
    # wd[ch][p, (9*cv+t)*128 + q] = delta_pq * wh_cv[ch*128+p, t]
    wd = np.zeros((3, 128, 27 * 128), np.float32)
    idx = np.arange(128)
    for ch in range(3):
        for cv in range(3):
            for t in range(9):
                wd[ch, idx, (9 * cv + t) * 128 + idx] = \
                    whs[cv][ch * 128 + idx, t]

    ind2 = np.zeros((2, 128), np.float32)
    ind2[0, 0:64] = 1.0
    ind2[1, 64:128] = 1.0

    wvp = np.zeros((C, NH * 65), np.float32)
    Wv = np.asarray(Wv, np.float32)
    for h in range(NH):
        wvp[:, h * 65:h * 65 + 64] = Wv[:, h * 64:(h + 1) * 64]

    return {
        "xb": xb,
        "wb": wb,
        "wd": wd.astype(bf),
        "wq": np.asarray(Wq, np.float32).astype(bf),
        "wk": np.asarray(Wk, np.float32).astype(bf),
        "wvp": wvp.astype(bf),
        "wpj": np.asarray(Wproj, np.float32).astype(bf),
        "ind2": ind2.astype(bf),
        "bpjW": _bpjw(np.asarray(bproj, np.float32)),
    }


def kernel(x, h, w, conv_q, conv_k, conv_v, bn_q, bn_k, bn_v, Wq, Wk, Wv,
           Wproj, bproj, _reps=1, _nc=None):
    B = x.shape[0]
    nc = _nc if _nc is not None else _build(_reps)
    hp = _host_prep(x, conv_q, conv_k, conv_v, bn_q, bn_k, bn_v, Wq, Wk, Wv,
                    Wproj, bproj)
    shared = {k: v for k, v in hp.items() if k != "xb"}
    in_maps = [dict(shared, xb=hp["xb"][b]) for b in range(B)]
    res = run_bass_kernel_spmd(nc, in_maps, core_ids=list(range(B)))
    out = np.stack([res.results[b]["out"] for b in range(B)], axis=0)
    return out.astype(np.float32)


# revision 32
# speedup vs baseline: 1.3354x; 1.3354x over previous
"""CvT attention block (depthwise conv QKV + MHA) on 8 Trainium2 NeuronCores,
data-parallel over batch.

v5 (head-pair packed attention + DVE-assisted softmax exp):
  - Phase A/B as v4: convs as diagonal-weight PE matmuls, projections bf16.
    KT zero-padded to 7x128 kv-tiles so every QK LDWEIGHTS is a 128-col FWL
    load; V-hat drops the ones-column (denominators via M=1 matmuls).
  - Phase C processes head PAIRS at l-chunks of 256: QK row-tiled
    (head h on PE rows 0-63, h+1 on 64-127, concurrent), AV col-tiled
    (head h -> PSUM partitions 0-63, h+1 -> 64-127, concurrent), so the
    128x128 array is fully used despite d=64 heads.
  - softmax exp is split across TWO engines: ACT exps S[:, 0:2048]
    (kv-tiles 0-3); a custom 2-instruction DVE op exps S[:, 2048:3584]
    (kv-tiles 4-6): p = deg-3 Taylor of e^(z*SCALE/64), then p^64 via 6
    squarings (~1e-4 poly err, bf16-out bound).
  - PSUM: one [128,3584] S tile (7 banks, ring via bufs=1) + one shared
    bank cycling AV-accum/denoms -> rb -> out-proj windows. 1-block
    software pipeline: block i emits QK(i), AV(i-1), rb/norm(i-2),
    out-proj/store pieces of the previous chunk, keeping the PE dense so
    the HAM clock gate stays at 8/8.
"""

import contextlib
import numpy as np
import ml_dtypes
from concourse import mybir
import concourse.bacc as bacc
import concourse.tile as tile
from concourse.bass_utils import run_bass_kernel_spmd
from concourse.dve_ops import DveOp, OPS, CUSTOM_DVE_SPECS, _SUB_OPCODE_FOR_NAME
from concourse.dve_spec import Spec, Src0, C0, C1, C2, One, sq, lower
from concourse.dve_uop import DveOpSpec

F32 = mybir.dt.float32
BF16 = mybir.dt.bfloat16
AFT = mybir.ActivationFunctionType

C = 384
T = 3136            # 56*56
TKV = 784           # 28*28
TKVP = 896          # 7*128 (zero-padded)
NH = 6
SCALE = C ** (-0.5)
EPS = 1e-5
XB = 3368           # 2 + 58*58 + 8 slack; data (r,c) at 2 + (1+r)*58 + 1+c

LS = 128
LC = [(i * LS, min(LS, T - i * LS)) for i in range((T + LS - 1) // LS)]  # 13
TT = [(j * 128, min(128, TKV - j * 128)) for j in range(7)]  # kv tiles
TAPS = [(t // 3 - 1, t % 3 - 1) for t in range(9)]

_CACHE = {}


# ---- custom DVE exp: exp(x*SCALE) = (deg3 e^(x*SCALE/64))^64 ---------------
def _register_dve(op_name, body, reference):
    spec = Spec(body=body, reference=reference)
    shas = {}
    for ver in ("v3", "v4"):
        uops = lower(spec, ver=ver)
        shas[ver] = DveOpSpec(name=op_name, opcode=0, uops=uops,
                              rd1_en=False).sha(ver)
    op = DveOp(op_name, spec, subdim=False, uops_sha=shas)
    if op_name not in _SUB_OPCODE_FOR_NAME:
        OPS.append(op)
        CUSTOM_DVE_SPECS[op_name] = spec
        _SUB_OPCODE_FOR_NAME[op_name] = max(_SUB_OPCODE_FOR_NAME.values()) + 1
        assert _SUB_OPCODE_FOR_NAME[op_name] < 0x20
    return op


_u = Src0 * C0
EXP_P1 = _register_dve(
    "ANT_EXP_P1", ((_u * C1 + One) * _u * C2 + One) * _u + One,
    lambda in0, in1, s0, s1, imm2: (
        ((in0 * s0 * s1 + 1.0) * (in0 * s0) * imm2 + 1.0) * (in0 * s0) + 1.0),
)
EXP_P2 = _register_dve(
    "ANT_EXP_P2", sq(sq(sq(sq(sq(sq(Src0)))))),
    lambda in0, in1, s0, s1, imm2: in0 ** 64,
)


def _emit(nc, tc, ctx, d, reps):
    pers = ctx.enter_context(tc.tile_pool(name="pers", bufs=1))

    wq = [pers.tile([128, C], BF16, tag=f"wq{i}", name=f"wq{i}") for i in range(3)]
    wk = [pers.tile([128, C], BF16, tag=f"wk{i}", name=f"wk{i}") for i in range(3)]
    wvp = [pers.tile([128, NH * 65], BF16, tag=f"wvp{i}", name=f"wvp{i}")
           for i in range(3)]
    wpj = [pers.tile([128, C], BF16, tag=f"wpj{i}", name=f"wpj{i}")
           for i in range(3)]
    wd = [pers.tile([128, 27 * 128], BF16, tag=f"wd{i}", name=f"wd{i}")
          for i in range(3)]
    ind2 = pers.tile([2, 128], BF16, tag="ind2", name="ind2")
    wb = [pers.tile([128, 30], F32, tag=f"wb{i}", name=f"wb{i}")
          for i in range(3)]
    bpjW = pers.tile([128, 1024], F32, tag="bpjW", name="bpjW")
    QT = [pers.tile([128, T], BF16, tag=f"QT{i}", name=f"QT{i}") for i in range(3)]
    KT = [pers.tile([128, TKVP], BF16, tag=f"KT{i}", name=f"KT{i}")
          for i in range(3)]
    Vh = [pers.tile([128, NH * 65], BF16, tag=f"Vh{i}", name=f"Vh{i}")
          for i in range(7)]

    nc.sync.dma_start(wd[0][:], d["wd"][0])
    nc.sync.dma_start(wb[0][:], d["wb"][0])
    for i in range(3):
        nc.vector.memset(KT[i][:, TKV:TKVP], 0.0)

    def _late_dmas():
        for i in range(1, 3):
            nc.sync.dma_start(wd[i][:], d["wd"][i])
            nc.sync.dma_start(wb[i][:], d["wb"][i])
        for i in range(3):
            nc.sync.dma_start(wq[i][:], d["wq"][i * 128:(i + 1) * 128, :])
            nc.sync.dma_start(wk[i][:], d["wk"][i * 128:(i + 1) * 128, :])
            nc.sync.dma_start(wvp[i][:], d["wvp"][i * 128:(i + 1) * 128, :])
            nc.sync.dma_start(wpj[i][:], d["wpj"][i * 128:(i + 1) * 128, :])
        nc.sync.dma_start(ind2[:], d["ind2"])
        nc.sync.dma_start(bpjW[:], d["bpjW"])

    for rep in range(reps):
        sfx = f"r{rep}"
        with contextlib.ExitStack() as ph:
            yp = ph.enter_context(tc.tile_pool(name="y" + sfx, bufs=1))
            yq = [yp.tile([128, T], BF16, tag=f"yq{i}", name=f"yq{i}")
                  for i in range(3)]
            yk = [yp.tile([128, TKV], BF16, tag=f"yk{i}", name=f"yk{i}")
                  for i in range(3)]
            yv = [yp.tile([128, TKV], BF16, tag=f"yv{i}", name=f"yv{i}")
                  for i in range(3)]
            xbs = [yp.tile([128, XB], BF16, tag=f"xb{i}", name=f"xb{i}")
                   for i in range(3)]
            ps = ph.enter_context(
                tc.tile_pool(name="ps" + sfx, bufs=1, space="PSUM"))
            # 4 + 1 + 1 + 1 banks; one spare
            Sb = ps.tile([128, 2048], F32, tag="Sb", name="Sb")
            rw = ps.tile([128, 512], F32, tag="rw", name="rw")
            pcv = ps.tile([128, 512], F32, tag="pcv", name="pcv")
            av = ps.tile([128, 512], F32, tag="av", name="av")
            psB0 = ps.tile([128, 512], F32, tag="psB0", name="psB0")

            def next_psB():
                return psB0
            cw = ph.enter_context(tc.tile_pool(name="cw" + sfx, bufs=2))

            for ch in range(3):
                nc.sync.dma_start(xbs[ch][:], d["xb"][ch * 128:(ch + 1) * 128, :])
                if rep == 0 and ch == 0:
                    _late_dmas()

            # ---- conv/proj piece emitters (PE work, dependency-light) ----
            def conv_q(ch, k):
                """q-conv rows 8k..8k+8 for channel block ch -> yq[ch]."""
                xb = xbs[ch]
                base = 2 + (1 + 8 * k) * 58
                for t, (di, dj) in enumerate(TAPS):
                    nc.tensor.matmul(pcv[:, 0:464],
                                     wd[ch][:, t * 128:(t + 1) * 128],
                                     xb[:, base + 58 * di + dj:
                                        base + 58 * di + dj + 464],
                                     start=(t == 0), stop=(t == 8))
                src = pcv[:, 0:464].rearrange(
                    "p (r c) -> p r c", c=58)[:, :, 1:57]
                dst = yq[ch][:, 448 * k:448 * (k + 1)].rearrange(
                    "p (r c) -> p r c", c=56)
                nc.scalar.activation(dst, src, AFT.Identity,
                                     bias=wb[ch][:, 27:28])

            def conv_kv(ch, cv, r0):
                xb = xbs[ch]
                x3 = xb[:, 2:2 + 3364].rearrange("p (r c) -> p r c", c=58)
                ykv = (yk if cv == 1 else yv)[ch]
                for t, (di, dj) in enumerate(TAPS):
                    mv = x3[:, 1 + 2 * r0 + di:1 + 2 * r0 + di + 28:2,
                            1 + dj:1 + dj + 56:2]
                    nc.tensor.matmul(
                        pcv[:, 0:392],
                        wd[ch][:, (9 * cv + t) * 128:(9 * cv + t + 1) * 128],
                        mv, start=(t == 0), stop=(t == 8))
                nc.scalar.activation(
                    ykv[:, r0 * 28:r0 * 28 + 392], pcv[:, 0:392], AFT.Identity,
                    bias=wb[ch][:, 27 + cv:28 + cv])

            def proj_q(co, g):
                """Q projection for output block co, l-cols [512g, 512g+ls)."""
                lo = 512 * g
                ls = min(512, T - lo)
                psB = next_psB()
                for ch in range(3):
                    nc.tensor.matmul(psB[:, 0:ls],
                                     wq[ch][:, co * 128:(co + 1) * 128],
                                     yq[ch][:, lo:lo + ls],
                                     start=(ch == 0), stop=(ch == 2))
                nc.scalar.activation(QT[co][:, lo:lo + ls], psB[:, 0:ls],
                                     AFT.Copy)

            def proj_k(co, g):
                to = 512 * g
                ts = min(512, TKV - to)
                psB = next_psB()
                for ch in range(3):
                    nc.tensor.matmul(psB[:, 0:ts],
                                     wk[ch][:, co * 128:(co + 1) * 128],
                                     yk[ch][:, to:to + ts],
                                     start=(ch == 0), stop=(ch == 2))
                nc.scalar.activation(KT[co][:, to:to + ts], psB[:, 0:ts],
                                     AFT.Copy)

            def proj_v(ti):
                to, ts = TT[ti]
                psB = next_psB()
                for ch in range(3):
                    nc.tensor.matmul(psB[0:ts, 0:NH * 65],
                                     yv[ch][:, to:to + ts], wvp[ch][:],
                                     start=(ch == 0), stop=(ch == 2))
                nc.scalar.activation(Vh[ti][0:ts, :], psB[0:ts, 0:NH * 65],
                                      AFT.Copy)
                nc.vector.memset(Vh[ti][0:ts, 64:NH * 65:65], 1.0)

            # ---- prologue: k/v side + enough of the q side for sb 0-11 ----
            for ch in range(3):
                for cv in (1, 2):
                    for r0 in (0, 14):
                        conv_kv(ch, cv, r0)
            for co in range(3):
                for g in range(2):
                    proj_k(co, g)
            for ti in range(7):
                proj_v(ti)
            for k in (0, 1):
                for ch in range(3):
                    conv_q(ch, k)
            for co in range(3):
                proj_q(co, 0)

            # remaining conv/proj work, popped one piece per 2 superblocks
            fill = []
            nextg = [1, 1, 1]
            for k in range(2, 7):
                for ch in range(3):
                    fill.append(lambda ch=ch, k=k: conv_q(ch, k))
                gr = (448 * (k + 1)) // 512
                for co in range(3):
                    while nextg[co] < min(gr, 7):
                        g = nextg[co]
                        fill.append(lambda co=co, g=g: proj_q(co, g))
                        nextg[co] += 1
            for co in range(3):
                while nextg[co] < 7:
                    g = nextg[co]
                    fill.append(lambda co=co, g=g: proj_q(co, g))
                    nextg[co] += 1

            # ---- phase C superblocks: (chunk, pair), AV lagging 1 block ----
            def qk_block(c, p):
                lo, ls = LC[c]
                for j in range(7):
                    for ho in range(2):
                        o = 896 * ho + 128 * j
                        nc.tensor.matmul(
                            Sb[0:128, o:o + ls],
                            KT[p][64 * ho:64 * ho + 64, 128 * j:128 * (j + 1)],
                            QT[p][64 * ho:64 * ho + 64, lo:lo + ls],
                            start=True, stop=True)
                et = cw.tile([128, 1792], BF16, tag="et", name="et")
                nc.scalar.activation(et[:], Sb[:, 0:1792], AFT.Exp,
                                     scale=float(SCALE))
                return {"c": c, "p": p, "et": et}

            def av_block(st):
                c, p = st["c"], st["p"]
                lo, ls = LC[c]
                OTb, rcfC = st["OTb_t"], st["rcfC_t"]
                et = st["et"]
                for ho in range(2):
                    h = 2 * p + ho
                    for j, (to, ts) in enumerate(TT):
                        nc.tensor.matmul(
                            av[0:65, 128 * ho:128 * ho + ls],
                            Vh[j][0:ts, 65 * h:65 * h + 65],
                            et[0:ts, 896 * ho + 128 * j:896 * ho + 128 * j + ls],
                            start=(j == 0), stop=(j == 6))
                for ho in range(2):
                    nc.vector.tensor_copy(
                        OTb[64 * ho:64 * ho + 64, 128 * p:128 * p + ls],
                        av[0:64, 128 * ho:128 * ho + ls])
                nc.vector.tensor_copy(rcfC[0:1, 256 * p:256 * p + 256],
                                      av[64:65, 0:256])

            def tail_enqueue(tq, c, OTb, rcfC):
                lo, ls = LC[c]
                lsz = min(128, ls)
                rcA = cw.tile([1, 768], F32, tag="rcA", name="rcA")
                rc2r = cw.tile([1, 768], BF16, tag="rc2r", name="rc2r")
                rc2v = cw.tile([2, 384], BF16, tag="rc2v", name="rc2v")
                osb = cw.tile([128, 512], F32, tag="osb", name="osb")

                def s_recip():
                    nc.vector.reciprocal_approx_fast(rcA[:], rcfC[:])
                    with nc.allow_low_precision(reason="bf16 softmax recip"):
                        nc.vector.tensor_copy(rc2r[:], rcA[:])
                    r3 = rc2r[0:1, :].rearrange("a (p h l) -> a p h l",
                                                h=2, l=128)
                    d3 = rc2v[:].rearrange("p (q l) -> p q l", l=128)
                    for ho in range(2):
                        nc.sync.dma_start(d3[ho:ho + 1], r3[:, :, ho, :])

                def s_norm(p):
                    nc.tensor.matmul(rw[:, 0:ls], ind2[:],
                                     rc2v[0:2, 128 * p:128 * p + ls],
                                     start=True, stop=True)
                    nc.vector.tensor_mul(OTb[:, 128 * p:128 * p + ls],
                                         OTb[:, 128 * p:128 * p + ls],
                                         rw[:, 0:ls])

                def s_win(cs):
                    w = rw[0:128, 128 * (cs + 1):128 * (cs + 2)]
                    for ch in range(3):
                        nc.tensor.matmul(
                            w[0:lsz, 0:128],
                            OTb[:, 128 * ch:128 * ch + lsz],
                            wpj[ch][:, 128 * cs:128 * cs + 128],
                            start=(ch == 0), stop=(ch == 2))
                    nc.vector.tensor_add(
                        osb[0:lsz, 128 * cs:128 * cs + 128],
                        w[0:lsz, 0:128],
                        bpjW[0:lsz, 128 * cs:128 * cs + 128])

                def s_store():
                    nc.sync.dma_start(d["out"][lo:lo + ls, :], osb[0:ls, 0:C])

                tq.append(s_recip)
                for p in range(3):
                    tq.append(lambda p=p: s_norm(p))

                def s_w01():
                    s_win(0)
                    s_win(1)

                def s_w2():
                    s_win(2)
                    s_store()
                tq.append(s_w01)
                tq.append(s_w2)

            blocks = [(c, p) for c in range(len(LC)) for p in range(3)]
            prev = None
            tq = []
            OTb = rcfC = None
            for i, (c, p) in enumerate(blocks):
                if p == 0:
                    OTb = cw.tile([128, 384], BF16, tag="OTb", name="OTb",
                                  bufs=3)
                    rcfC = cw.tile([1, 768], F32, tag="rcfC", name="rcfC",
                                   bufs=3)
                st = qk_block(c, p)
                st["OTb_t"], st["rcfC_t"] = OTb, rcfC
                if tq:
                    tq.pop(0)()
                if prev is not None:
                    av_block(prev)
                    if prev["p"] == 2:
                        tail_enqueue(tq, prev["c"], prev["OTb_t"],
                                     prev["rcfC_t"])
                if tq:
                    tq.pop(0)()
                if i % 2 == 0 and fill:
                    fill.pop(0)()
                prev = st

            av_block(prev)
            tail_enqueue(tq, prev["c"], prev["OTb_t"], prev["rcfC_t"])
            for s in fill:
                s()
            for s in tq:
                s()


def _build(reps=1):
    if reps in _CACHE:
        return _CACHE[reps]
    nc = bacc.Bacc("TRN2", target_bir_lowering=False, debug=False)
    d = {
        "xb": nc.dram_tensor("xb", [C, XB], BF16, kind="ExternalInput").ap(),
        "wb": nc.dram_tensor("wb", [3, 128, 30], F32, kind="ExternalInput").ap(),
        "wd": nc.dram_tensor("wd", [3, 128, 27 * 128], BF16,
                             kind="ExternalInput").ap(),
        "wq": nc.dram_tensor("wq", [C, C], BF16, kind="ExternalInput").ap(),
        "wk": nc.dram_tensor("wk", [C, C], BF16, kind="ExternalInput").ap(),
        "wvp": nc.dram_tensor("wvp", [C, NH * 65], BF16,
                              kind="ExternalInput").ap(),
        "wpj": nc.dram_tensor("wpj", [C, C], BF16, kind="ExternalInput").ap(),
        "ind2": nc.dram_tensor("ind2", [2, 128], BF16,
                               kind="ExternalInput").ap(),
        "bpjW": nc.dram_tensor("bpjW", [128, 1024], F32,
                               kind="ExternalInput").ap(),
        "out": nc.dram_tensor("out", [T, C], F32, kind="ExternalOutput").ap(),
    }
    with tile.TileContext(nc) as tc:
        with contextlib.ExitStack() as ctx:
            _emit(nc, tc, ctx, d, reps)
    nc.compile()
    _CACHE[reps] = nc
    return nc


def _bpjw(bproj):
    w = np.zeros((128, 1024), np.float32)
    for k in range(2):
        w[:, k * 512:k * 512 + C] = bproj[None, :]
    return w


def _host_prep(x, conv_q, conv_k, conv_v, bn_q, bn_k, bn_v, Wq, Wk, Wv,
               Wproj, bproj):
    bf = ml_dtypes.bfloat16
    B = x.shape[0]
    x = np.asarray(x, np.float32)
    # 58x58 zero-padded bf16 image: data (r,c) at col 2 + (1+r)*58 + 1+c
    xb = np.zeros((B, C, XB), bf)
    xi = np.ascontiguousarray(x.transpose(0, 2, 1)).reshape(B, C, 56, 56)
    xb3 = xb[:, :, 2:2 + 3364].reshape(B, C, 58, 58)
    xb3[:, :, 1:57, 1:57] = xi.astype(bf)

    wb = np.zeros((3, 128, 30), np.float32)
    whs = []
    for cv, (w, bn) in enumerate(((conv_q, bn_q), (conv_k, bn_k),
                                  (conv_v, bn_v))):
        g, b, m, v = [np.asarray(bn[i], np.float64) for i in range(4)]
        a = g / np.sqrt(v + EPS)
        bias = (b - m * a).astype(np.float32)
        wh = (np.asarray(w, np.float64).reshape(C, 9) * a[:, None]).astype(
            np.float32)
        whs.append(wh)
        for ch in range(3):
            wb[ch, :, 9 * cv:9 * cv + 9] = wh[ch * 128:(ch + 1) * 128]
            wb[ch, :, 27 + cv] = bias[ch * 128:(ch + 1) * 128]

    # diag-packed conv weights for the PE:
    # wd[ch][p, (9*cv+t)*128 + q] = delta_pq * wh_cv[ch*128+p, t]
    wd = np.zeros((3, 128, 27 * 128), np.float32)
    idx = np.arange(128)
    for ch in range(3):
        for cv in range(3):
            for t in range(9):
                wd[ch, idx, (9 * cv + t) * 128 + idx] = \
                    whs[cv][ch * 128 + idx, t]

    ind2 = np.zeros((2, 128), np.float32)
    ind2[0, 0:64] = 1.0
    ind2[1, 64:128] = 1.0

    wvp = np.zeros((C, NH * 65), np.float32)
    Wv = np.asarray(Wv, np.float32)
    for h in range(NH):
        wvp[:, h * 65:h * 65 + 64] = Wv[:, h * 64:(h + 1) * 64]

    return {
        "xb": xb,
        "wb": wb,
        "wd": wd.astype(bf),
        "wq": np.asarray(Wq, np.float32).astype(bf),
        "wk": np.asarray(Wk, np.float32).astype(bf),
        "wvp": wvp.astype(bf),
        "wpj": np.asarray(Wproj, np.float32).astype(bf),
        "ind2": ind2.astype(bf),
        "bpjW": _bpjw(np.asarray(bproj, np.float32)),
    }


def kernel(x, h, w, conv_q, conv_k, conv_v, bn_q, bn_k, bn_v, Wq, Wk, Wv,
           Wproj, bproj, _reps=1, _nc=None):
    B = x.shape[0]
    nc = _nc if _nc is not None else _build(_reps)
    hp = _host_prep(x, conv_q, conv_k, conv_v, bn_q, bn_k, bn_v, Wq, Wk, Wv,
                    Wproj, bproj)
    shared = {k: v for k, v in hp.items() if k != "xb"}
    in_maps = [dict(shared, xb=hp["xb"][b]) for b in range(B)]
    res = run_bass_kernel_spmd(nc, in_maps, core_ids=list(range(B)))
    out = np.stack([res.results[b]["out"] for b in range(B)], axis=0)
    return out.astype(np.float32)


# revision 34
# speedup vs baseline: 1.3591x; 1.0178x over previous
"""CvT attention block (depthwise conv QKV + MHA) on 8 Trainium2 NeuronCores,
data-parallel over batch.

v6 (interleaved conv/proj filler + head-pair attention at ls=128):
  - K/V side (strided convs, K/V projections, zero-padded KT to 7x128
    kv-tiles, V-hat with ones column) runs as a short prologue; the q-side
    work (q-conv diagonal-matmul pieces and Q-projection blocks) is held
    back as a list of dependency-light PE "filler" pieces.
  - Phase C iterates (l-chunk of 128) x (head pair): QK uses row-tiled
    matmul pairs (head h on PE rows 0-63, h+1 on 64-127, concurrent), one
    wide ACT exp call per block, AV with the M=65 ones-column trick
    (denominators for free), normalization via an ind2 broadcast matmul,
    out-projection in 3x128-col windows. One filler piece is emitted every
    other block so the PE stays dense across the exp/DVE latency, keeping
    the HAM clock gate at 8/8 for most of the run.
  - PSUM (8 banks): S [128,2048] (14 QK units), av [128,512] (AV + denom
    row), rw [128,512] (rb + 3 out-proj windows), conv scratch, proj
    scratch. Hazards are tile-granular, so each pipelined resource owns
    its own tile.
  - conv-extract (fused BN bias) and projection PSUM->SBUF copies run on
    the Scalar engine (activation Identity-with-bias / Copy), unloading
    the Vector engine, which handles softmax denominators, normalization
    and output bias.
"""

import contextlib
import numpy as np
import ml_dtypes
from concourse import mybir
import concourse.bacc as bacc
import concourse.tile as tile
from concourse.bass_utils import run_bass_kernel_spmd
from concourse.dve_ops import DveOp, OPS, CUSTOM_DVE_SPECS, _SUB_OPCODE_FOR_NAME
from concourse.dve_spec import Spec, Src0, C0, C1, C2, One, sq, lower
from concourse.dve_uop import DveOpSpec

F32 = mybir.dt.float32
BF16 = mybir.dt.bfloat16
AFT = mybir.ActivationFunctionType

C = 384
T = 3136            # 56*56
TKV = 784           # 28*28
TKVP = 896          # 7*128 (zero-padded)
NH = 6
SCALE = C ** (-0.5)
EPS = 1e-5
XB = 3368           # 2 + 58*58 + 8 slack; data (r,c) at 2 + (1+r)*58 + 1+c

LS = 128
LC = [(i * LS, min(LS, T - i * LS)) for i in range((T + LS - 1) // LS)]  # 13
TT = [(j * 128, min(128, TKV - j * 128)) for j in range(7)]  # kv tiles
TAPS = [(t // 3 - 1, t % 3 - 1) for t in range(9)]

_CACHE = {}


# ---- custom DVE exp: exp(x*SCALE) = (deg3 e^(x*SCALE/64))^64 ---------------
def _register_dve(op_name, body, reference):
    spec = Spec(body=body, reference=reference)
    shas = {}
    for ver in ("v3", "v4"):
        uops = lower(spec, ver=ver)
        shas[ver] = DveOpSpec(name=op_name, opcode=0, uops=uops,
                              rd1_en=False).sha(ver)
    op = DveOp(op_name, spec, subdim=False, uops_sha=shas)
    if op_name not in _SUB_OPCODE_FOR_NAME:
        OPS.append(op)
        CUSTOM_DVE_SPECS[op_name] = spec
        _SUB_OPCODE_FOR_NAME[op_name] = max(_SUB_OPCODE_FOR_NAME.values()) + 1
        assert _SUB_OPCODE_FOR_NAME[op_name] < 0x20
    return op


_u = Src0 * C0
EXP_P1 = _register_dve(
    "ANT_EXP_P1", ((_u * C1 + One) * _u * C2 + One) * _u + One,
    lambda in0, in1, s0, s1, imm2: (
        ((in0 * s0 * s1 + 1.0) * (in0 * s0) * imm2 + 1.0) * (in0 * s0) + 1.0),
)
EXP_P2 = _register_dve(
    "ANT_EXP_P2", sq(sq(sq(sq(sq(sq(Src0)))))),
    lambda in0, in1, s0, s1, imm2: in0 ** 64,
)


def _emit(nc, tc, ctx, d, reps):
    pers = ctx.enter_context(tc.tile_pool(name="pers", bufs=1))

    wq = [pers.tile([128, C], BF16, tag=f"wq{i}", name=f"wq{i}") for i in range(3)]
    wk = [pers.tile([128, C], BF16, tag=f"wk{i}", name=f"wk{i}") for i in range(3)]
    wvp = [pers.tile([128, NH * 65], BF16, tag=f"wvp{i}", name=f"wvp{i}")
           for i in range(3)]
    wpj = [pers.tile([128, C], BF16, tag=f"wpj{i}", name=f"wpj{i}")
           for i in range(3)]
    wd = [pers.tile([128, 27 * 128], BF16, tag=f"wd{i}", name=f"wd{i}")
          for i in range(3)]
    ind2 = pers.tile([2, 128], BF16, tag="ind2", name="ind2")
    wb = [pers.tile([128, 30], F32, tag=f"wb{i}", name=f"wb{i}")
          for i in range(3)]
    bpjW = pers.tile([128, 1024], F32, tag="bpjW", name="bpjW")
    QT = [pers.tile([128, T], BF16, tag=f"QT{i}", name=f"QT{i}") for i in range(3)]
    KT = [pers.tile([128, TKVP], BF16, tag=f"KT{i}", name=f"KT{i}")
          for i in range(3)]
    Vh = [pers.tile([128, NH * 65], BF16, tag=f"Vh{i}", name=f"Vh{i}")
          for i in range(7)]

    nc.sync.dma_start(wd[0][:], d["wd"][0])
    nc.sync.dma_start(wb[0][:], d["wb"][0])
    for i in range(3):
        nc.vector.memset(KT[i][:, TKV:TKVP], 0.0)

    def _late_dmas():
        for i in range(1, 3):
            nc.sync.dma_start(wd[i][:], d["wd"][i])
            nc.sync.dma_start(wb[i][:], d["wb"][i])
        for i in range(3):
            nc.sync.dma_start(wq[i][:], d["wq"][i * 128:(i + 1) * 128, :])
            nc.sync.dma_start(wk[i][:], d["wk"][i * 128:(i + 1) * 128, :])
            nc.sync.dma_start(wvp[i][:], d["wvp"][i * 128:(i + 1) * 128, :])
            nc.sync.dma_start(wpj[i][:], d["wpj"][i * 128:(i + 1) * 128, :])
        nc.sync.dma_start(ind2[:], d["ind2"])
        nc.sync.dma_start(bpjW[:], d["bpjW"])

    for rep in range(reps):
        sfx = f"r{rep}"
        with contextlib.ExitStack() as ph:
            yp = ph.enter_context(tc.tile_pool(name="y" + sfx, bufs=1))
            yq = [yp.tile([128, T], BF16, tag=f"yq{i}", name=f"yq{i}")
                  for i in range(3)]
            yk = [yp.tile([128, TKV], BF16, tag=f"yk{i}", name=f"yk{i}")
                  for i in range(3)]
            yv = [yp.tile([128, TKV], BF16, tag=f"yv{i}", name=f"yv{i}")
                  for i in range(3)]
            xbs = [yp.tile([128, XB], BF16, tag=f"xb{i}", name=f"xb{i}")
                   for i in range(3)]
            ps = ph.enter_context(
                tc.tile_pool(name="ps" + sfx, bufs=1, space="PSUM"))
            # 4 + 1 + 1 + 1 banks; one spare
            Sb = ps.tile([128, 2048], F32, tag="Sb", name="Sb")
            rw = ps.tile([128, 512], F32, tag="rw", name="rw")
            pcv = ps.tile([128, 512], F32, tag="pcv", name="pcv")
            av = ps.tile([128, 512], F32, tag="av", name="av")
            psB0 = ps.tile([128, 512], F32, tag="psB0", name="psB0")
            npb = [0]

            def next_psB():
                # round-robin conv/proj filler scratch over two banks so a
                # piece never waits on the previous piece's extract
                npb[0] += 1
                return psB0 if npb[0] % 2 else pcv
            cw = ph.enter_context(tc.tile_pool(name="cw" + sfx, bufs=2))

            for ch in range(3):
                nc.sync.dma_start(xbs[ch][:], d["xb"][ch * 128:(ch + 1) * 128, :])
                if rep == 0 and ch == 0:
                    _late_dmas()

            # ---- conv/proj piece emitters (PE work, dependency-light) ----
            def conv_q(ch, k):
                """q-conv rows 8k..8k+8 for channel block ch -> yq[ch]."""
                xb = xbs[ch]
                pcv = next_psB()
                base = 2 + (1 + 8 * k) * 58
                for t, (di, dj) in enumerate(TAPS):
                    nc.tensor.matmul(pcv[:, 0:464],
                                     wd[ch][:, t * 128:(t + 1) * 128],
                                     xb[:, base + 58 * di + dj:
                                        base + 58 * di + dj + 464],
                                     start=(t == 0), stop=(t == 8))
                src = pcv[:, 0:464].rearrange(
                    "p (r c) -> p r c", c=58)[:, :, 1:57]
                dst = yq[ch][:, 448 * k:448 * (k + 1)].rearrange(
                    "p (r c) -> p r c", c=56)
                nc.scalar.activation(dst, src, AFT.Identity,
                                     bias=wb[ch][:, 27:28])

            def conv_kv(ch, cv, r0):
                xb = xbs[ch]
                pcv = next_psB()
                x3 = xb[:, 2:2 + 3364].rearrange("p (r c) -> p r c", c=58)
                ykv = (yk if cv == 1 else yv)[ch]
                for t, (di, dj) in enumerate(TAPS):
                    mv = x3[:, 1 + 2 * r0 + di:1 + 2 * r0 + di + 28:2,
                            1 + dj:1 + dj + 56:2]
                    nc.tensor.matmul(
                        pcv[:, 0:392],
                        wd[ch][:, (9 * cv + t) * 128:(9 * cv + t + 1) * 128],
                        mv, start=(t == 0), stop=(t == 8))
                nc.scalar.activation(
                    ykv[:, r0 * 28:r0 * 28 + 392], pcv[:, 0:392], AFT.Identity,
                    bias=wb[ch][:, 27 + cv:28 + cv])

            def proj_q(co, g):
                """Q projection for output block co, l-cols [512g, 512g+ls)."""
                lo = 512 * g
                ls = min(512, T - lo)
                psB = next_psB()
                for ch in range(3):
                    nc.tensor.matmul(psB[:, 0:ls],
                                     wq[ch][:, co * 128:(co + 1) * 128],
                                     yq[ch][:, lo:lo + ls],
                                     start=(ch == 0), stop=(ch == 2))
                nc.scalar.activation(QT[co][:, lo:lo + ls], psB[:, 0:ls],
                                     AFT.Copy)

            def proj_k(co, g):
                to = 512 * g
                ts = min(512, TKV - to)
                psB = next_psB()
                for ch in range(3):
                    nc.tensor.matmul(psB[:, 0:ts],
                                     wk[ch][:, co * 128:(co + 1) * 128],
                                     yk[ch][:, to:to + ts],
                                     start=(ch == 0), stop=(ch == 2))
                nc.scalar.activation(KT[co][:, to:to + ts], psB[:, 0:ts],
                                     AFT.Copy)

            def proj_v(ti):
                to, ts = TT[ti]
                psB = next_psB()
                for ch in range(3):
                    nc.tensor.matmul(psB[0:ts, 0:NH * 65],
                                     yv[ch][:, to:to + ts], wvp[ch][:],
                                     start=(ch == 0), stop=(ch == 2))
                nc.scalar.activation(Vh[ti][0:ts, :], psB[0:ts, 0:NH * 65],
                                      AFT.Copy)
                nc.vector.memset(Vh[ti][0:ts, 64:NH * 65:65], 1.0)

            # ---- prologue: k/v side + enough of the q side for sb 0-11 ----
            for ch in range(3):
                for cv in (1, 2):
                    for r0 in (0, 14):
                        conv_kv(ch, cv, r0)
            for co in range(3):
                for g in range(2):
                    proj_k(co, g)
            for ti in range(7):
                proj_v(ti)
            for k in (0, 1):
                for ch in range(3):
                    conv_q(ch, k)
            for co in range(3):
                proj_q(co, 0)

            # remaining conv/proj work, popped one piece per 2 superblocks
            fill = []
            nextg = [1, 1, 1]
            for k in range(2, 7):
                for ch in range(3):
                    fill.append(lambda ch=ch, k=k: conv_q(ch, k))
                gr = (448 * (k + 1)) // 512
                for co in range(3):
                    while nextg[co] < min(gr, 7):
                        g = nextg[co]
                        fill.append(lambda co=co, g=g: proj_q(co, g))
                        nextg[co] += 1
            for co in range(3):
                while nextg[co] < 7:
                    g = nextg[co]
                    fill.append(lambda co=co, g=g: proj_q(co, g))
                    nextg[co] += 1

            # ---- phase C superblocks: (chunk, pair), AV lagging 1 block ----
            def qk_block(c, p):
                lo, ls = LC[c]
                for j in range(7):
                    for ho in range(2):
                        o = 896 * ho + 128 * j
                        nc.tensor.matmul(
                            Sb[0:128, o:o + ls],
                            KT[p][64 * ho:64 * ho + 64, 128 * j:128 * (j + 1)],
                            QT[p][64 * ho:64 * ho + 64, lo:lo + ls],
                            start=True, stop=True)
                et = cw.tile([128, 1792], BF16, tag="et", name="et")
                nc.scalar.activation(et[:], Sb[:, 0:1792], AFT.Exp,
                                     scale=float(SCALE))
                return {"c": c, "p": p, "et": et}

            def av_block(st):
                c, p = st["c"], st["p"]
                lo, ls = LC[c]
                OTb, rcfC = st["OTb_t"], st["rcfC_t"]
                et = st["et"]
                for ho in range(2):
                    h = 2 * p + ho
                    for j, (to, ts) in enumerate(TT):
                        nc.tensor.matmul(
                            av[0:65, 128 * ho:128 * ho + ls],
                            Vh[j][0:ts, 65 * h:65 * h + 65],
                            et[0:ts, 896 * ho + 128 * j:896 * ho + 128 * j + ls],
                            start=(j == 0), stop=(j == 6))
                for ho in range(2):
                    nc.vector.tensor_copy(
                        OTb[64 * ho:64 * ho + 64, 128 * p:128 * p + ls],
                        av[0:64, 128 * ho:128 * ho + ls])
                nc.vector.tensor_copy(rcfC[0:1, 256 * p:256 * p + 256],
                                      av[64:65, 0:256])

            def tail_enqueue(tq, c, OTb, rcfC):
                lo, ls = LC[c]
                lsz = min(128, ls)
                rcA = cw.tile([1, 768], F32, tag="rcA", name="rcA")
                rc2r = cw.tile([1, 768], BF16, tag="rc2r", name="rc2r")
                rc2v = cw.tile([2, 384], BF16, tag="rc2v", name="rc2v")
                osb = cw.tile([128, 512], F32, tag="osb", name="osb")

                def s_recip():
                    nc.vector.reciprocal_approx_fast(rcA[:], rcfC[:])
                    with nc.allow_low_precision(reason="bf16 softmax recip"):
                        nc.vector.tensor_copy(rc2r[:], rcA[:])
                    r3 = rc2r[0:1, :].rearrange("a (p h l) -> a p h l",
                                                h=2, l=128)
                    d3 = rc2v[:].rearrange("p (q l) -> p q l", l=128)
                    for ho in range(2):
                        nc.sync.dma_start(d3[ho:ho + 1], r3[:, :, ho, :])

                def s_norm(p):
                    nc.tensor.matmul(rw[:, 0:ls], ind2[:],
                                     rc2v[0:2, 128 * p:128 * p + ls],
                                     start=True, stop=True)
                    nc.vector.tensor_mul(OTb[:, 128 * p:128 * p + ls],
                                         OTb[:, 128 * p:128 * p + ls],
                                         rw[:, 0:ls])

                def s_win(cs):
                    w = rw[0:128, 128 * (cs + 1):128 * (cs + 2)]
                    for ch in range(3):
                        nc.tensor.matmul(
                            w[0:lsz, 0:128],
                            OTb[:, 128 * ch:128 * ch + lsz],
                            wpj[ch][:, 128 * cs:128 * cs + 128],
                            start=(ch == 0), stop=(ch == 2))
                    nc.vector.tensor_add(
                        osb[0:lsz, 128 * cs:128 * cs + 128],
                        w[0:lsz, 0:128],
                        bpjW[0:lsz, 128 * cs:128 * cs + 128])

                def s_store():
                    nc.sync.dma_start(d["out"][lo:lo + ls, :], osb[0:ls, 0:C])

                tq.append(s_recip)
                for p in range(3):
                    tq.append(lambda p=p: s_norm(p))

                def s_w01():
                    s_win(0)
                    s_win(1)

                def s_w2():
                    s_win(2)
                    s_store()
                tq.append(s_w01)
                tq.append(s_w2)

            blocks = [(c, p) for c in range(len(LC)) for p in range(3)]
            prev = None
            tq = []
            OTb = rcfC = None
            for i, (c, p) in enumerate(blocks):
                if p == 0:
                    OTb = cw.tile([128, 384], BF16, tag="OTb", name="OTb",
                                  bufs=3)
                    rcfC = cw.tile([1, 768], F32, tag="rcfC", name="rcfC",
                                   bufs=3)
                st = qk_block(c, p)
                st["OTb_t"], st["rcfC_t"] = OTb, rcfC
                if tq:
                    tq.pop(0)()
                if prev is not None:
                    av_block(prev)
                    if prev["p"] == 2:
                        tail_enqueue(tq, prev["c"], prev["OTb_t"],
                                     prev["rcfC_t"])
                if tq:
                    tq.pop(0)()
                if i % 2 == 0 and fill:
                    fill.pop(0)()
                prev = st

            av_block(prev)
            tail_enqueue(tq, prev["c"], prev["OTb_t"], prev["rcfC_t"])
            for s in fill:
                s()
            for s in tq:
                s()


def _build(reps=1):
    if reps in _CACHE:
        return _CACHE[reps]
    nc = bacc.Bacc("TRN2", target_bir_lowering=False, debug=False)
    d = {
        "xb": nc.dram_tensor("xb", [C, XB], BF16, kind="ExternalInput").ap(),
        "wb": nc.dram_tensor("wb", [3, 128, 30], F32, kind="ExternalInput").ap(),
        "wd": nc.dram_tensor("wd", [3, 128, 27 * 128], BF16,
                             kind="ExternalInput").ap(),
        "wq": nc.dram_tensor("wq", [C, C], BF16, kind="ExternalInput").ap(),
        "wk": nc.dram_tensor("wk", [C, C], BF16, kind="ExternalInput").ap(),
        "wvp": nc.dram_tensor("wvp", [C, NH * 65], BF16,
                              kind="ExternalInput").ap(),
        "wpj": nc.dram_tensor("wpj", [C, C], BF16, kind="ExternalInput").ap(),
        "ind2": nc.dram_tensor("ind2", [2, 128], BF16,
                               kind="ExternalInput").ap(),
        "bpjW": nc.dram_tensor("bpjW", [128, 1024], F32,
                               kind="ExternalInput").ap(),
        "out": nc.dram_tensor("out", [T, C], F32, kind="ExternalOutput").ap(),
    }
    with tile.TileContext(nc) as tc:
        with contextlib.ExitStack() as ctx:
            _emit(nc, tc, ctx, d, reps)
    nc.compile()
    _CACHE[reps] = nc
    return nc


def _bpjw(bproj):
    w = np.zeros((128, 1024), np.float32)
    for k in range(2):
        w[:, k * 512:k * 512 + C] = bproj[None, :]
    return w


def _host_prep(x, conv_q, conv_k, conv_v, bn_q, bn_k, bn_v, Wq, Wk, Wv,
               Wproj, bproj):
    bf = ml_dtypes.bfloat16
    B = x.shape[0]
    x = np.asarray(x, np.float32)
    # 58x58 zero-padded bf16 image: data (r,c) at col 2 + (1+r)*58 + 1+c
    xb = np.zeros((B, C, XB), bf)
    xi = np.ascontiguousarray(x.transpose(0, 2, 1)).reshape(B, C, 56, 56)
    xb3 = xb[:, :, 2:2 + 3364].reshape(B, C, 58, 58)
    xb3[:, :, 1:57, 1:57] = xi.astype(bf)

    wb = np.zeros((3, 128, 30), np.float32)
    whs = []
    for cv, (w, bn) in enumerate(((conv_q, bn_q), (conv_k, bn_k),
                                  (conv_v, bn_v))):
        g, b, m, v = [np.asarray(bn[i], np.float64) for i in range(4)]
        a = g / np.sqrt(v + EPS)
        bias = (b - m * a).astype(np.float32)
        wh = (np.asarray(w, np.float64).reshape(C, 9) * a[:, None]).astype(
            np.float32)
        whs.append(wh)
        for ch in range(3):
            wb[ch, :, 9 * cv:9 * cv + 9] = wh[ch * 128:(ch + 1) * 128]
            wb[ch, :, 27 + cv] = bias[ch * 128:(ch + 1) * 128]

    # diag-packed conv weights for the PE:
    # wd[ch][p, (9*cv+t)*128 + q] = delta_pq * wh_cv[ch*128+p, t]
    wd = np.zeros((3, 128, 27 * 128), np.float32)
    idx = np.arange(128)
    for ch in range(3):
        for cv in range(3):
            for t in range(9):
                wd[ch, idx, (9 * cv + t) * 128 + idx] = \
                    whs[cv][ch * 128 + idx, t]

    ind2 = np.zeros((2, 128), np.float32)
    ind2[0, 0:64] = 1.0
    ind2[1, 64:128] = 1.0

    wvp = np.zeros((C, NH * 65), np.float32)
    Wv = np.asarray(Wv, np.float32)
    for h in range(NH):
        wvp[:, h * 65:h * 65 + 64] = Wv[:, h * 64:(h + 1) * 64]

    return {
        "xb": xb,
        "wb": wb,
        "wd": wd.astype(bf),
        "wq": np.asarray(Wq, np.float32).astype(bf),
        "wk": np.asarray(Wk, np.float32).astype(bf),
        "wvp": wvp.astype(bf),
        "wpj": np.asarray(Wproj, np.float32).astype(bf),
        "ind2": ind2.astype(bf),
        "bpjW": _bpjw(np.asarray(bproj, np.float32)),
    }


def kernel(x, h, w, conv_q, conv_k, conv_v, bn_q, bn_k, bn_v, Wq, Wk, Wv,
           Wproj, bproj, _reps=1, _nc=None):
    B = x.shape[0]
    nc = _nc if _nc is not None else _build(_reps)
    hp = _host_prep(x, conv_q, conv_k, conv_v, bn_q, bn_k, bn_v, Wq, Wk, Wv,
                    Wproj, bproj)
    shared = {k: v for k, v in hp.items() if k != "xb"}
    in_maps = [dict(shared, xb=hp["xb"][b]) for b in range(B)]
    res = run_bass_kernel_spmd(nc, in_maps, core_ids=list(range(B)))
    out = np.stack([res.results[b]["out"] for b in range(B)], axis=0)
    return out.astype(np.float32)


# revision 36
# speedup vs baseline: 1.3909x; 1.0234x over previous
"""CvT attention block (depthwise conv QKV + MHA) on 8 Trainium2 NeuronCores,
data-parallel over batch.

v6 (interleaved conv/proj filler + head-pair attention at ls=128):
  - K/V side (strided convs, K/V projections, zero-padded KT to 7x128
    kv-tiles, V-hat with ones column) runs as a short prologue; the q-side
    work (q-conv diagonal-matmul pieces and Q-projection blocks) is held
    back as a list of dependency-light PE "filler" pieces.
  - Phase C iterates (l-chunk of 128) x (head pair): QK uses row-tiled
    matmul pairs (head h on PE rows 0-63, h+1 on 64-127, concurrent), one
    wide ACT exp call per block, AV with the M=65 ones-column trick
    (denominators for free), normalization via an ind2 broadcast matmul,
    out-projection in 3x128-col windows. One filler piece is emitted every
    other block so the PE stays dense across the exp/DVE latency, keeping
    the HAM clock gate at 8/8 for most of the run.
  - PSUM (8 banks): S [128,2048] (14 QK units), av [128,512] (AV + denom
    row), rw [128,512] (rb + 3 out-proj windows), conv scratch, proj
    scratch. Hazards are tile-granular, so each pipelined resource owns
    its own tile.
  - conv-extract (fused BN bias) and projection PSUM->SBUF copies run on
    the Scalar engine (activation Identity-with-bias / Copy), unloading
    the Vector engine, which handles softmax denominators, normalization
    and output bias.
"""

import contextlib
import numpy as np
import ml_dtypes
from concourse import mybir
import concourse.bacc as bacc
import concourse.tile as tile
from concourse.bass_utils import run_bass_kernel_spmd
from concourse.dve_ops import DveOp, OPS, CUSTOM_DVE_SPECS, _SUB_OPCODE_FOR_NAME
from concourse.dve_spec import Spec, Src0, C0, C1, C2, One, sq, lower
from concourse.dve_uop import DveOpSpec

F32 = mybir.dt.float32
BF16 = mybir.dt.bfloat16
AFT = mybir.ActivationFunctionType

C = 384
T = 3136            # 56*56
TKV = 784           # 28*28
TKVP = 896          # 7*128 (zero-padded)
NH = 6
SCALE = C ** (-0.5)
EPS = 1e-5
XB = 3368           # 2 + 58*58 + 8 slack; data (r,c) at 2 + (1+r)*58 + 1+c

LS = 128
LC = [(i * LS, min(LS, T - i * LS)) for i in range((T + LS - 1) // LS)]  # 13
TT = [(j * 128, min(128, TKV - j * 128)) for j in range(7)]  # kv tiles
TAPS = [(t // 3 - 1, t % 3 - 1) for t in range(9)]

_CACHE = {}


# ---- custom DVE exp: exp(x*SCALE) = (deg3 e^(x*SCALE/64))^64 ---------------
def _register_dve(op_name, body, reference):
    spec = Spec(body=body, reference=reference)
    shas = {}
    for ver in ("v3", "v4"):
        uops = lower(spec, ver=ver)
        shas[ver] = DveOpSpec(name=op_name, opcode=0, uops=uops,
                              rd1_en=False).sha(ver)
    op = DveOp(op_name, spec, subdim=False, uops_sha=shas)
    if op_name not in _SUB_OPCODE_FOR_NAME:
        OPS.append(op)
        CUSTOM_DVE_SPECS[op_name] = spec
        _SUB_OPCODE_FOR_NAME[op_name] = max(_SUB_OPCODE_FOR_NAME.values()) + 1
        assert _SUB_OPCODE_FOR_NAME[op_name] < 0x20
    return op


_u = Src0 * C0
EXP_P1 = _register_dve(
    "ANT_EXP_P1", ((_u * C1 + One) * _u * C2 + One) * _u + One,
    lambda in0, in1, s0, s1, imm2: (
        ((in0 * s0 * s1 + 1.0) * (in0 * s0) * imm2 + 1.0) * (in0 * s0) + 1.0),
)
EXP_P2 = _register_dve(
    "ANT_EXP_P2", sq(sq(sq(sq(sq(sq(Src0)))))),
    lambda in0, in1, s0, s1, imm2: in0 ** 64,
)


def _emit(nc, tc, ctx, d, reps):
    pers = ctx.enter_context(tc.tile_pool(name="pers", bufs=1))

    wq = [pers.tile([128, C], BF16, tag=f"wq{i}", name=f"wq{i}") for i in range(3)]
    wk = [pers.tile([128, C], BF16, tag=f"wk{i}", name=f"wk{i}") for i in range(3)]
    wvp = [pers.tile([128, NH * 65], BF16, tag=f"wvp{i}", name=f"wvp{i}")
           for i in range(3)]
    wpj = [pers.tile([128, C], BF16, tag=f"wpj{i}", name=f"wpj{i}")
           for i in range(3)]
    wd = [pers.tile([128, 27 * 128], BF16, tag=f"wd{i}", name=f"wd{i}")
          for i in range(3)]
    ind2 = pers.tile([2, 128], BF16, tag="ind2", name="ind2")
    wb = [pers.tile([128, 30], F32, tag=f"wb{i}", name=f"wb{i}")
          for i in range(3)]
    bpjW = pers.tile([128, 1024], F32, tag="bpjW", name="bpjW")
    QT = [pers.tile([128, T], BF16, tag=f"QT{i}", name=f"QT{i}") for i in range(3)]
    KT = [pers.tile([128, TKVP], BF16, tag=f"KT{i}", name=f"KT{i}")
          for i in range(3)]
    Vh = [pers.tile([128, NH * 65], BF16, tag=f"Vh{i}", name=f"Vh{i}")
          for i in range(7)]

    nc.sync.dma_start(wd[0][:], d["wd"][0])
    nc.sync.dma_start(wb[0][:], d["wb"][0])
    for i in range(3):
        nc.vector.memset(KT[i][:, TKV:TKVP], 0.0)

    def _late_dmas():
        for i in range(1, 3):
            nc.sync.dma_start(wd[i][:], d["wd"][i])
            nc.sync.dma_start(wb[i][:], d["wb"][i])
        for i in range(3):
            nc.sync.dma_start(wq[i][:], d["wq"][i * 128:(i + 1) * 128, :])
            nc.sync.dma_start(wk[i][:], d["wk"][i * 128:(i + 1) * 128, :])
            nc.sync.dma_start(wvp[i][:], d["wvp"][i * 128:(i + 1) * 128, :])
            nc.sync.dma_start(wpj[i][:], d["wpj"][i * 128:(i + 1) * 128, :])
        nc.sync.dma_start(ind2[:], d["ind2"])
        nc.sync.dma_start(bpjW[:], d["bpjW"])

    for rep in range(reps):
        sfx = f"r{rep}"
        with contextlib.ExitStack() as ph:
            yp = ph.enter_context(tc.tile_pool(name="y" + sfx, bufs=1))
            yq = [yp.tile([128, T], BF16, tag=f"yq{i}", name=f"yq{i}")
                  for i in range(3)]
            yk = [yp.tile([128, TKV], BF16, tag=f"yk{i}", name=f"yk{i}")
                  for i in range(3)]
            yv = [yp.tile([128, TKV], BF16, tag=f"yv{i}", name=f"yv{i}")
                  for i in range(3)]
            xbs = [yp.tile([128, XB], BF16, tag=f"xb{i}", name=f"xb{i}")
                   for i in range(3)]
            ps = ph.enter_context(
                tc.tile_pool(name="ps" + sfx, bufs=1, space="PSUM"))
            # 4 + 1 + 1 + 1 banks; one spare
            Sb = ps.tile([128, 2048], F32, tag="Sb", name="Sb")
            rw = ps.tile([128, 512], F32, tag="rw", name="rw")
            pcv = ps.tile([128, 512], F32, tag="pcv", name="pcv")
            av = ps.tile([128, 512], F32, tag="av", name="av")
            psB0 = ps.tile([128, 512], F32, tag="psB0", name="psB0")
            npb = [0]

            def next_psB():
                # round-robin conv/proj filler scratch over two banks so a
                # piece never waits on the previous piece's extract
                npb[0] += 1
                return psB0 if npb[0] % 2 else pcv
            cw = ph.enter_context(tc.tile_pool(name="cw" + sfx, bufs=2))

            for ch in range(3):
                nc.sync.dma_start(xbs[ch][:], d["xb"][ch * 128:(ch + 1) * 128, :])

            # ---- conv/proj piece emitters (PE work, dependency-light) ----
            def conv_q(ch, k):
                """q-conv rows 8k..8k+8 for channel block ch -> yq[ch]."""
                xb = xbs[ch]
                pcv = next_psB()
                base = 2 + (1 + 8 * k) * 58
                for t, (di, dj) in enumerate(TAPS):
                    nc.tensor.matmul(pcv[:, 0:464],
                                     wd[ch][:, t * 128:(t + 1) * 128],
                                     xb[:, base + 58 * di + dj:
                                        base + 58 * di + dj + 464],
                                     start=(t == 0), stop=(t == 8))
                src = pcv[:, 0:464].rearrange(
                    "p (r c) -> p r c", c=58)[:, :, 1:57]
                dst = yq[ch][:, 448 * k:448 * (k + 1)].rearrange(
                    "p (r c) -> p r c", c=56)
                nc.scalar.activation(dst, src, AFT.Identity,
                                     bias=wb[ch][:, 27:28])

            def conv_kv(ch, cv, r0):
                xb = xbs[ch]
                pcv = next_psB()
                x3 = xb[:, 2:2 + 3364].rearrange("p (r c) -> p r c", c=58)
                ykv = (yk if cv == 1 else yv)[ch]
                for t, (di, dj) in enumerate(TAPS):
                    mv = x3[:, 1 + 2 * r0 + di:1 + 2 * r0 + di + 28:2,
                            1 + dj:1 + dj + 56:2]
                    nc.tensor.matmul(
                        pcv[:, 0:392],
                        wd[ch][:, (9 * cv + t) * 128:(9 * cv + t + 1) * 128],
                        mv, start=(t == 0), stop=(t == 8))
                nc.scalar.activation(
                    ykv[:, r0 * 28:r0 * 28 + 392], pcv[:, 0:392], AFT.Identity,
                    bias=wb[ch][:, 27 + cv:28 + cv])

            def proj_q(co, g):
                """Q projection for output block co, l-cols [512g, 512g+ls)."""
                lo = 512 * g
                ls = min(512, T - lo)
                psB = next_psB()
                for ch in range(3):
                    nc.tensor.matmul(psB[:, 0:ls],
                                     wq[ch][:, co * 128:(co + 1) * 128],
                                     yq[ch][:, lo:lo + ls],
                                     start=(ch == 0), stop=(ch == 2))
                nc.scalar.activation(QT[co][:, lo:lo + ls], psB[:, 0:ls],
                                     AFT.Copy)

            def proj_k(co, g):
                to = 512 * g
                ts = min(512, TKV - to)
                psB = next_psB()
                for ch in range(3):
                    nc.tensor.matmul(psB[:, 0:ts],
                                     wk[ch][:, co * 128:(co + 1) * 128],
                                     yk[ch][:, to:to + ts],
                                     start=(ch == 0), stop=(ch == 2))
                nc.scalar.activation(KT[co][:, to:to + ts], psB[:, 0:ts],
                                     AFT.Copy)

            def proj_v(ti):
                to, ts = TT[ti]
                psB = next_psB()
                for ch in range(3):
                    nc.tensor.matmul(psB[0:ts, 0:NH * 65],
                                     yv[ch][:, to:to + ts], wvp[ch][:],
                                     start=(ch == 0), stop=(ch == 2))
                nc.scalar.activation(Vh[ti][0:ts, :], psB[0:ts, 0:NH * 65],
                                      AFT.Copy)
                nc.vector.memset(Vh[ti][0:ts, 64:NH * 65:65], 1.0)

            # ---- prologue: k/v side + enough of the q side for sb 0-11 ----
            for ch in range(3):
                for cv in (1, 2):
                    for r0 in (0, 14):
                        conv_kv(ch, cv, r0)
                if rep == 0 and ch == 0:
                    _late_dmas()
            for co in range(3):
                for g in range(2):
                    proj_k(co, g)
            for ti in range(7):
                proj_v(ti)
            for k in (0, 1):
                for ch in range(3):
                    conv_q(ch, k)
            for co in range(3):
                proj_q(co, 0)

            # remaining conv/proj work, popped one piece per 2 superblocks
            fill = []
            nextg = [1, 1, 1]
            for k in range(2, 7):
                for ch in range(3):
                    fill.append(lambda ch=ch, k=k: conv_q(ch, k))
                gr = (448 * (k + 1)) // 512
                for co in range(3):
                    while nextg[co] < min(gr, 7):
                        g = nextg[co]
                        fill.append(lambda co=co, g=g: proj_q(co, g))
                        nextg[co] += 1
            for co in range(3):
                while nextg[co] < 7:
                    g = nextg[co]
                    fill.append(lambda co=co, g=g: proj_q(co, g))
                    nextg[co] += 1

            # ---- phase C superblocks: (chunk, pair), AV lagging 1 block ----
            def qk_block(c, p):
                lo, ls = LC[c]
                for j in range(7):
                    for ho in range(2):
                        o = 896 * ho + 128 * j
                        nc.tensor.matmul(
                            Sb[0:128, o:o + ls],
                            KT[p][64 * ho:64 * ho + 64, 128 * j:128 * (j + 1)],
                            QT[p][64 * ho:64 * ho + 64, lo:lo + ls],
                            start=True, stop=True)
                et = cw.tile([128, 1792], BF16, tag="et", name="et")
                nc.scalar.activation(et[:], Sb[:, 0:1792], AFT.Exp,
                                     scale=float(SCALE))
                return {"c": c, "p": p, "et": et}

            def av_block(st):
                c, p = st["c"], st["p"]
                lo, ls = LC[c]
                OTb, rcfC = st["OTb_t"], st["rcfC_t"]
                et = st["et"]
                for ho in range(2):
                    h = 2 * p + ho
                    for j, (to, ts) in enumerate(TT):
                        nc.tensor.matmul(
                            av[0:65, 128 * ho:128 * ho + ls],
                            Vh[j][0:ts, 65 * h:65 * h + 65],
                            et[0:ts, 896 * ho + 128 * j:896 * ho + 128 * j + ls],
                            start=(j == 0), stop=(j == 6))
                for ho in range(2):
                    nc.vector.tensor_copy(
                        OTb[64 * ho:64 * ho + 64, 128 * p:128 * p + ls],
                        av[0:64, 128 * ho:128 * ho + ls])
                nc.vector.tensor_copy(rcfC[0:1, 256 * p:256 * p + 256],
                                      av[64:65, 0:256])

            def tail_enqueue(tq, c, OTb, rcfC):
                lo, ls = LC[c]
                lsz = min(128, ls)
                rcA = cw.tile([1, 768], F32, tag="rcA", name="rcA")
                rc2r = cw.tile([1, 768], BF16, tag="rc2r", name="rc2r")
                rc2v = cw.tile([2, 384], BF16, tag="rc2v", name="rc2v")
                osb = cw.tile([128, 512], F32, tag="osb", name="osb")

                def s_recip():
                    nc.vector.reciprocal_approx_fast(rcA[:], rcfC[:])
                    with nc.allow_low_precision(reason="bf16 softmax recip"):
                        nc.vector.tensor_copy(rc2r[:], rcA[:])
                    r3 = rc2r[0:1, :].rearrange("a (p h l) -> a p h l",
                                                h=2, l=128)
                    d3 = rc2v[:].rearrange("p (q l) -> p q l", l=128)
                    for ho in range(2):
                        nc.sync.dma_start(d3[ho:ho + 1], r3[:, :, ho, :])

                def s_norm(p):
                    nc.tensor.matmul(rw[:, 0:ls], ind2[:],
                                     rc2v[0:2, 128 * p:128 * p + ls],
                                     start=True, stop=True)
                    nc.vector.tensor_mul(OTb[:, 128 * p:128 * p + ls],
                                         OTb[:, 128 * p:128 * p + ls],
                                         rw[:, 0:ls])

                def s_win(cs):
                    w = rw[0:128, 128 * (cs + 1):128 * (cs + 2)]
                    for ch in range(3):
                        nc.tensor.matmul(
                            w[0:lsz, 0:128],
                            OTb[:, 128 * ch:128 * ch + lsz],
                            wpj[ch][:, 128 * cs:128 * cs + 128],
                            start=(ch == 0), stop=(ch == 2))
                    nc.vector.tensor_add(
                        osb[0:lsz, 128 * cs:128 * cs + 128],
                        w[0:lsz, 0:128],
                        bpjW[0:lsz, 128 * cs:128 * cs + 128])

                def s_store():
                    nc.sync.dma_start(d["out"][lo:lo + ls, :], osb[0:ls, 0:C])

                def s_n12():
                    s_norm(1)
                    s_norm(2)

                def s_w01():
                    s_win(0)
                    s_win(1)

                def s_w2():
                    s_win(2)
                    s_store()
                # a spacer slot between the last normalize and the first
                # out-proj window gives the DVE mul time to retire, so the
                # window matmuls don't open a PE gap every chunk
                tq.extend([s_recip, lambda: s_norm(0), s_n12,
                           lambda: None, s_w01, s_w2])

            blocks = [(c, p) for c in range(len(LC)) for p in range(3)]
            prev = None
            tq = []
            OTb = rcfC = None
            for i, (c, p) in enumerate(blocks):
                if p == 0:
                    OTb = cw.tile([128, 384], BF16, tag="OTb", name="OTb",
                                  bufs=3)
                    rcfC = cw.tile([1, 768], F32, tag="rcfC", name="rcfC",
                                   bufs=3)
                st = qk_block(c, p)
                st["OTb_t"], st["rcfC_t"] = OTb, rcfC
                if tq:
                    tq.pop(0)()
                if prev is not None:
                    av_block(prev)
                    if prev["p"] == 2:
                        tail_enqueue(tq, prev["c"], prev["OTb_t"],
                                     prev["rcfC_t"])
                if tq:
                    tq.pop(0)()
                if i % 2 == 0 and fill:
                    fill.pop(0)()
                prev = st

            av_block(prev)
            tail_enqueue(tq, prev["c"], prev["OTb_t"], prev["rcfC_t"])
            for s in fill:
                s()
            for s in tq:
                s()


def _build(reps=1):
    if reps in _CACHE:
        return _CACHE[reps]
    nc = bacc.Bacc("TRN2", target_bir_lowering=False, debug=False)
    d = {
        "xb": nc.dram_tensor("xb", [C, XB], BF16, kind="ExternalInput").ap(),
        "wb": nc.dram_tensor("wb", [3, 128, 30], F32, kind="ExternalInput").ap(),
        "wd": nc.dram_tensor("wd", [3, 128, 27 * 128], BF16,
                             kind="ExternalInput").ap(),
        "wq": nc.dram_tensor("wq", [C, C], BF16, kind="ExternalInput").ap(),
        "wk": nc.dram_tensor("wk", [C, C], BF16, kind="ExternalInput").ap(),
        "wvp": nc.dram_tensor("wvp", [C, NH * 65], BF16,
                              kind="ExternalInput").ap(),
        "wpj": nc.dram_tensor("wpj", [C, C], BF16, kind="ExternalInput").ap(),
        "ind2": nc.dram_tensor("ind2", [2, 128], BF16,
                               kind="ExternalInput").ap(),
        "bpjW": nc.dram_tensor("bpjW", [128, 1024], F32,
                               kind="ExternalInput").ap(),
        "out": nc.dram_tensor("out", [T, C], F32, kind="ExternalOutput").ap(),
    }
    with tile.TileContext(nc) as tc:
        with contextlib.ExitStack() as ctx:
            _emit(nc, tc, ctx, d, reps)
    nc.compile()
    _CACHE[reps] = nc
    return nc


def _bpjw(bproj):
    w = np.zeros((128, 1024), np.float32)
    for k in range(2):
        w[:, k * 512:k * 512 + C] = bproj[None, :]
    return w


def _host_prep(x, conv_q, conv_k, conv_v, bn_q, bn_k, bn_v, Wq, Wk, Wv,
               Wproj, bproj):
    bf = ml_dtypes.bfloat16
    B = x.shape[0]
    x = np.asarray(x, np.float32)
    # 58x58 zero-padded bf16 image: data (r,c) at col 2 + (1+r)*58 + 1+c
    xb = np.zeros((B, C, XB), bf)
    xi = np.ascontiguousarray(x.transpose(0, 2, 1)).reshape(B, C, 56, 56)
    xb3 = xb[:, :, 2:2 + 3364].reshape(B, C, 58, 58)
    xb3[:, :, 1:57, 1:57] = xi.astype(bf)

    wb = np.zeros((3, 128, 30), np.float32)
    whs = []
    for cv, (w, bn) in enumerate(((conv_q, bn_q), (conv_k, bn_k),
                                  (conv_v, bn_v))):
        g, b, m, v = [np.asarray(bn[i], np.float64) for i in range(4)]
        a = g / np.sqrt(v + EPS)
        bias = (b - m * a).astype(np.float32)
        wh = (np.asarray(w, np.float64).reshape(C, 9) * a[:, None]).astype(
            np.float32)
        whs.append(wh)
        for ch in range(3):
            wb[ch, :, 9 * cv:9 * cv + 9] = wh[ch * 128:(ch + 1) * 128]
            wb[ch, :, 27 + cv] = bias[ch * 128:(ch + 1) * 128]

    # diag-packed conv weights for the PE:
    # wd[ch][p, (9*cv+t)*128 + q] = delta_pq * wh_cv[ch*128+p, t]
    wd = np.zeros((3, 128, 27 * 128), np.float32)
    idx = np.arange(128)
    for ch in range(3):
        for cv in range(3):
            for t in range(9):
                wd[ch, idx, (9 * cv + t) * 128 + idx] = \
                    whs[cv][ch * 128 + idx, t]

    ind2 = np.zeros((2, 128), np.float32)
    ind2[0, 0:64] = 1.0
    ind2[1, 64:128] = 1.0

    wvp = np.zeros((C, NH * 65), np.float32)
    Wv = np.asarray(Wv, np.float32)
    for h in range(NH):
        wvp[:, h * 65:h * 65 + 64] = Wv[:, h * 64:(h + 1) * 64]

    return {
        "xb": xb,
        "wb": wb,
        "wd": wd.astype(bf),
        "wq": np.asarray(Wq, np.float32).astype(bf),
        "wk": np.asarray(Wk, np.float32).astype(bf),
        "wvp": wvp.astype(bf),
        "wpj": np.asarray(Wproj, np.float32).astype(bf),
        "ind2": ind2.astype(bf),
        "bpjW": _bpjw(np.asarray(bproj, np.float32)),
    }


def kernel(x, h, w, conv_q, conv_k, conv_v, bn_q, bn_k, bn_v, Wq, Wk, Wv,
           Wproj, bproj, _reps=1, _nc=None):
    B = x.shape[0]
    nc = _nc if _nc is not None else _build(_reps)
    hp = _host_prep(x, conv_q, conv_k, conv_v, bn_q, bn_k, bn_v, Wq, Wk, Wv,
                    Wproj, bproj)
    shared = {k: v for k, v in hp.items() if k != "xb"}
    in_maps = [dict(shared, xb=hp["xb"][b]) for b in range(B)]
    res = run_bass_kernel_spmd(nc, in_maps, core_ids=list(range(B)))
    out = np.stack([res.results[b]["out"] for b in range(B)], axis=0)
    return out.astype(np.float32)
